# revision 1
# baseline (speedup 1.0000x reference)
"""LCSA (local convolutional sparse attention) Trainium2 Bass kernel.

Problem: B=2, S=2048, D=1024, H=8 heads, E=128 head width, KW=16 kernel width,
per-head dilations [1,1,2,2,4,4,8,8].

Sharding: pure data-parallel over (batch, sequence): core c handles batch c//4,
sequence chunk (c%4)*512..+512. Each core loads a 640-token haloed slice of x
(64-token halo each side, zero-padded at batch edges; padding reproduces the
reference's "invalid position -> bias" semantics exactly since k(0)=bk, v(0)=bv).

Device algorithm per core (all in fp32):
  - x arrives pre-transposed [D=1024, 640] (host does the transpose for free).
  - qT[h] = (Wq[h].T @ xT) [E,512] and kT[h] [E,640] via PE with W chunks
    stationary; v = xT.T @ Wv_allheads [640, H*E] with xT chunks stationary.
  - Per (query-tile i of 128, head h): logits = qT_tile.T @ kT_window -> [128,256]
    (full 256-key span; additive -30000 mask keeps only the 16 dilated window
    positions), softmax along free dim (DVE max / ACT exp+rowsum / DVE recip),
    score transposed via PE, attnT = v_span.T-chunks @ scoreT, out accumulated
    over heads: out[i] = sum_h attnT[h].T @ Wo[h] (Wo pre-scaled by E**-0.5).
"""

import numpy as np

B, S, D, H, E, KW = 2, 2048, 1024, 8, 128, 16
HALO = 64          # covers max offset d*(KW-1)//2 = 60 for d=8
CHUNK = 512        # query tokens per core
SPAN = CHUNK + 2 * HALO   # 640 = 5*128 kv tokens per core
NST = SPAN // 128  # 5 sequence tiles
NQT = CHUNK // 128 # 4 query tiles
NC_ = 8            # cores
DC = D // 128      # 8 contraction chunks
MASKVAL = -30000.0

_CACHE: dict = {}


def _build_nc():
    from contextlib import ExitStack

    import concourse.bacc as bacc
    import concourse.tile as tile
    from concourse import mybir
    from concourse.masks import make_identity

    F32 = mybir.dt.float32
    AX = mybir.AxisListType.X
    AF = mybir.ActivationFunctionType

    nc = bacc.Bacc("TRN2", target_bir_lowering=False, debug=False, num_devices=1)

    xt_d = nc.dram_tensor("xt", [D, SPAN], F32, kind="ExternalInput").ap()
    wq_d = nc.dram_tensor("wq", [H, D, E], F32, kind="ExternalInput").ap()
    wk_d = nc.dram_tensor("wk", [H, D, E], F32, kind="ExternalInput").ap()
    wvr_d = nc.dram_tensor("wvr", [D, H * E], F32, kind="ExternalInput").ap()
    wos_d = nc.dram_tensor("wos", [H, E, D], F32, kind="ExternalInput").ap()
    mk_d = nc.dram_tensor("mk", [H, 128, 256], F32, kind="ExternalInput").ap()
    bqt_d = nc.dram_tensor("bqt", [E, H], F32, kind="ExternalInput").ap()
    bkt_d = nc.dram_tensor("bkt", [E, H], F32, kind="ExternalInput").ap()
    bvr_d = nc.dram_tensor("bvr", [1, H * E], F32, kind="ExternalInput").ap()
    bor_d = nc.dram_tensor("bor", [1, D], F32, kind="ExternalInput").ap()
    out_d = nc.dram_tensor("out", [CHUNK, D], F32, kind="ExternalOutput").ap()

    with tile.TileContext(nc) as tc, ExitStack() as ctx:
        const_p = ctx.enter_context(tc.tile_pool(name="const", bufs=1))
        big_p = ctx.enter_context(tc.tile_pool(name="big", bufs=1))
        wqk_p = ctx.enter_context(tc.tile_pool(name="wqk", bufs=2))
        sm_p = ctx.enter_context(tc.tile_pool(name="sm", bufs=3))
        smv_p = ctx.enter_context(tc.tile_pool(name="smv", bufs=6))
        at_p = ctx.enter_context(tc.tile_pool(name="atsb", bufs=3))
        ob_p = ctx.enter_context(tc.tile_pool(name="ob", bufs=2))
        ps_pj = ctx.enter_context(tc.tile_pool(name="ps_pj", bufs=2, space="PSUM"))
        ps_lg = ctx.enter_context(tc.tile_pool(name="ps_lg", bufs=1, space="PSUM"))
        ps_st = ctx.enter_context(tc.tile_pool(name="ps_st", bufs=1, space="PSUM"))
        ps_at = ctx.enter_context(tc.tile_pool(name="ps_at", bufs=2, space="PSUM"))
        ps_ou = ctx.enter_context(tc.tile_pool(name="ps_ou", bufs=2, space="PSUM"))

        # constants
        ident = const_p.tile([128, 128], F32)
        make_identity(nc, ident)
        ones = const_p.tile([1, 128], F32)
        nc.gpsimd.memset(ones, 1.0)

        # resident loads
        xt_sb = big_p.tile([128, DC, SPAN], F32)
        nc.sync.dma_start(xt_sb, xt_d.rearrange("(c p) s -> p c s", p=128))
        bqt_sb = big_p.tile([128, H], F32)
        nc.sync.dma_start(bqt_sb, bqt_d)
        bkt_sb = big_p.tile([128, H], F32)
        nc.sync.dma_start(bkt_sb, bkt_d)
        bvr_sb = big_p.tile([1, H * E], F32)
        nc.sync.dma_start(bvr_sb, bvr_d)
        bor_sb = big_p.tile([1, D], F32)
        nc.sync.dma_start(bor_sb, bor_d)
        mk_sb = big_p.tile([128, H, 256], F32)
        nc.sync.dma_start(mk_sb, mk_d.rearrange("h p t -> p h t"))
        wvr_sb = big_p.tile([128, DC, H * E], F32)
        nc.sync.dma_start(wvr_sb, wvr_d.rearrange("(c p) n -> p c n", p=128))
        wos_sb = big_p.tile([128, H, D], F32)
        nc.sync.dma_start(wos_sb, wos_d.rearrange("h e d -> e h d"))

        # persistent projection outputs
        qT_sb = big_p.tile([128, H, CHUNK], F32)   # [e, h, s]
        kT_sb = big_p.tile([128, H, SPAN], F32)    # [e, h, s]
        v_sb = big_p.tile([128, NST, H * E], F32)  # [s, tile, h*E+e]

        # ---- phase 1a: q/k projections per head (W chunks stationary) ----
        for h in range(H):
            wq_sb = wqk_p.tile([128, DC, E], F32, tag="wq")
            nc.sync.dma_start(wq_sb, wq_d[h].rearrange("(c p) e -> p c e", p=128))
            wk_sb = wqk_p.tile([128, DC, E], F32, tag="wk")
            nc.sync.dma_start(wk_sb, wk_d[h].rearrange("(c p) e -> p c e", p=128))

            qp = ps_pj.tile([128, 512], F32, tag="pj")
            for c in range(DC):
                nc.tensor.matmul(qp, wq_sb[:, c, :], xt_sb[:, c, HALO:HALO + CHUNK],
                                 start=(c == 0), stop=(c == DC - 1))
            nc.vector.tensor_scalar_add(qT_sb[:, h, :], qp, bqt_sb[:, h:h + 1])

            for half in range(2):
                kp = ps_pj.tile([128, 512], F32, tag="pj")
                sl = slice(320 * half, 320 * (half + 1))
                for c in range(DC):
                    nc.tensor.matmul(kp[:, 0:320], wk_sb[:, c, :], xt_sb[:, c, sl],
                                     start=(c == 0), stop=(c == DC - 1))
                nc.vector.tensor_scalar_add(kT_sb[:, h, sl], kp[:, 0:320],
                                            bkt_sb[:, h:h + 1])

        # ---- phase 1b: v projection, all heads at once (xT chunks stationary) ----
        for j in range(NST):
            for half in range(2):
                vp = ps_pj.tile([128, 512], F32, tag="pj")
                nsl = slice(512 * half, 512 * (half + 1))
                for c in range(DC):
                    nc.tensor.matmul(vp, xt_sb[:, c, 128 * j:128 * (j + 1)],
                                     wvr_sb[:, c, nsl], start=(c == 0), stop=False)
                nc.tensor.matmul(vp, ones, bvr_sb[:, nsl], start=False, stop=True)
                nc.vector.tensor_copy(v_sb[:, j, nsl], vp)

        # ---- phase 2: attention + output projection per query tile ----
        for i in range(NQT):
            ou0 = ps_ou.tile([128, 512], F32, tag="ou")
            ou1 = ps_ou.tile([128, 512], F32, tag="ou")
            for h in range(H):
                lg = ps_lg.tile([128, 256], F32, tag="lg")
                nc.tensor.matmul(lg, qT_sb[:, h, 128 * i:128 * (i + 1)],
                                 kT_sb[:, h, 128 * i:128 * i + 256],
                                 start=True, stop=True)
                lm = sm_p.tile([128, 256], F32, tag="lm")
                nc.vector.tensor_add(lm, lg, mk_sb[:, h, :])
                nm = smv_p.tile([128, 1], F32, tag="nm")
                nc.vector.reduce_max(nm, lm, axis=AX, negate=True)
                ex = sm_p.tile([128, 256], F32, tag="ex")
                se = smv_p.tile([128, 1], F32, tag="se")
                nc.scalar.activation(ex, lm, AF.Exp, bias=nm, scale=1.0, accum_out=se)
                rc = smv_p.tile([128, 1], F32, tag="rc")
                nc.vector.reciprocal(rc, se)
                sc = sm_p.tile([128, 256], F32, tag="sc")
                nc.vector.tensor_scalar_mul(sc, ex, rc)

                st = ps_st.tile([128, 256], F32, tag="st")
                nc.tensor.transpose(st[:, 0:128], sc[:, 0:128], ident)
                nc.tensor.transpose(st[:, 128:256], sc[:, 128:256], ident)
                sct = sm_p.tile([128, 256], F32, tag="sct")
                nc.vector.tensor_copy(sct, st)

                at = ps_at.tile([128, 128], F32, tag="at")
                nc.tensor.matmul(at, v_sb[:, i, E * h:E * (h + 1)], sct[:, 0:128],
                                 start=True, stop=False)
                nc.tensor.matmul(at, v_sb[:, i + 1, E * h:E * (h + 1)], sct[:, 128:256],
                                 start=False, stop=True)
                ats = at_p.tile([128, 128], F32, tag="ats")
                nc.vector.tensor_copy(ats, at)

                nc.tensor.matmul(ou0, ats, wos_sb[:, h, 0:512],
                                 start=(h == 0), stop=False)
                nc.tensor.matmul(ou1, ats, wos_sb[:, h, 512:1024],
                                 start=(h == 0), stop=False)
            nc.tensor.matmul(ou0, ones, bor_sb[:, 0:512], start=False, stop=True)
            nc.tensor.matmul(ou1, ones, bor_sb[:, 512:1024], start=False, stop=True)

            ob = ob_p.tile([128, D], F32, tag="ob")
            nc.scalar.copy(ob[:, 0:512], ou0)
            nc.scalar.copy(ob[:, 512:1024], ou1)
            nc.sync.dma_start(out_d[128 * i:128 * (i + 1), :], ob)

    nc.compile()
    return nc


def _host_prep(x, Wq, bq, Wk, bk, Wv, bv, Wo, bo, dilations):
    f = np.float32
    x = np.asarray(x, f)
    x_pad = np.zeros((B, S + 2 * HALO, D), f)
    x_pad[:, HALO:HALO + S] = x

    wvr = np.ascontiguousarray(
        np.asarray(Wv, f).transpose(1, 0, 2).reshape(D, H * E))
    wos = np.ascontiguousarray(np.asarray(Wo, f) * np.float32(E) ** f(-0.5))
    bqt = np.ascontiguousarray(np.asarray(bq, f).T)      # [E, H]
    bkt = np.ascontiguousarray(np.asarray(bk, f).T)
    bvr = np.ascontiguousarray(np.asarray(bv, f).reshape(1, H * E))
    bor = np.ascontiguousarray(np.asarray(bo, f).reshape(1, D))

    dil = np.asarray(dilations).astype(np.int64)
    masks = np.full((H, 128, 256), MASKVAL, f)
    s_i = np.arange(128)[:, None]
    t_i = np.arange(256)[None, :]
    for h in range(H):
        d = int(dil[h])
        off = (d * (KW - 1)) // 2
        delta = t_i - s_i - HALO + off
        win = (delta >= 0) & (delta <= (KW - 1) * d) & (delta % d == 0)
        masks[h][win] = 0.0

    shared = {
        "wq": np.ascontiguousarray(np.asarray(Wq, f)),
        "wk": np.ascontiguousarray(np.asarray(Wk, f)),
        "wvr": wvr, "wos": wos, "mk": masks,
        "bqt": bqt, "bkt": bkt, "bvr": bvr, "bor": bor,
    }
    in_maps = []
    for c in range(NC_):
        b, idx = divmod(c, 4)
        xt = np.ascontiguousarray(x_pad[b, idx * CHUNK: idx * CHUNK + SPAN].T)
        in_maps.append({"xt": xt, **shared})
    return in_maps


def kernel(x, Wq, bq, Wk, bk, Wv, bv, Wo, bo, dilations):
    from concourse.bass_utils import run_bass_kernel_spmd

    if "nc" not in _CACHE:
        _CACHE["nc"] = _build_nc()
    nc = _CACHE["nc"]

    in_maps = _host_prep(x, Wq, bq, Wk, bk, Wv, bv, Wo, bo, dilations)
    res = run_bass_kernel_spmd(nc, in_maps, core_ids=list(range(NC_)))

    out = np.empty((B, S, D), np.float32)
    for c in range(NC_):
        b, idx = divmod(c, 4)
        out[b, idx * CHUNK:(idx + 1) * CHUNK] = res.results[c]["out"]
    return out
